# revision 23
# baseline (speedup 1.0000x reference)
"""Trainium2 Bass kernel for nn_GCNWithAttention_sign (GNN message passing).

Reference computation (N=8192 nodes, K=32 neighbors, D=128, H=256):
    A[i]   = x[i] @ W1[:D]  + b1          (self part of layer 1)
    B[j]   = x[j] @ W1[D:]                (neighbor part of layer 1)
    h1     = relu(A[i] + B[j])            per (i, k) pair, j = chosen[i, k]
    h2     = relu(h1 @ W2 + b2)
    mlp    = h2 @ W3 + b3                 -> [N, K]
    scores = softmax(b * |mlp|, axis=k)
    w      = mlp * scores
    Y_pred = sum_k (t - e_hat)[j] * w
    pairwise[i, chosen[i, k]] = w (last k wins), diagonal zeroed

Sharding: nodes row-partitioned across 8 cores (1024 rows each). x, t,
e_hat, and MLP weights are replicated. Each core computes its 1024x8192
block of `pairwise` (zero-fill + element scatter) and its Y_pred slice.

On-chip pipeline per core (transposed data flow, partition dim = hidden):
  - B table for all 8192 nodes computed on PE (fp32r), stored bf16 in SBUF.
  - dma_gather (SBUF source, transpose=True) pulls neighbor B rows already
    transposed: BT_g[h % 128, h // 128, q] = B[j(q), h].
  - h1T = relu(BT_g + A'T)  on DVE (bf16).
  - h2T = relu(W2^T h1T + b2) : 4 bf16 matmuls per tile + ACT bias-relu.
  - mlp: block-diagonal W3 matmuls accumulate all 32 k into one PSUM tile
    [32, 1024] so the per-(i,k) result lands in a dense, cheap layout.
  - softmax/weights on 128-row tiles; Y_pred via fused multiply-reduce.
  - pairwise: dense zero-fill (32 MB) + 4-byte indirect scatter of the
    32 nonzeros per row (k ascending so duplicate columns keep the last k).
"""

import os
import sys

import numpy as np

for _p in ("/opt/trn_rl_repo", "/root/.axon_site/_ro/trn_rl_repo"):
    if os.path.isdir(_p) and _p not in sys.path:
        sys.path.append(_p)

import ml_dtypes

import concourse.bass as bass
import concourse.bacc as bacc
import concourse.tile as tile
from concourse import mybir
from concourse.tile import add_dep_helper
from concourse import library_config

bf16 = ml_dtypes.bfloat16

N, K, D, H = 8192, 32, 128, 256
NCORES = 8
NL = N // NCORES          # local nodes per core (1024)
P = 128
NT = NL // P              # row tiles per core (8)
QTOT = NL * K             # pairs per core (32768)
GB = 8                    # gather batches
QB = QTOT // GB           # idxs per gather batch (8192)
IB = 512                  # i-block size for compute tiles
F32 = mybir.dt.float32
F32R = mybir.dt.float32r
BF16 = mybir.dt.bfloat16
I16 = mybir.dt.int16
I32 = mybir.dt.int32


PHASES = set(os.environ.get("KPHASES", "gather,mlp,post,scatter").split(","))
POSTLVL = int(os.environ.get("POSTLVL", "9"))


def _trace_kernel(nc, tc, ins, outs):
    """Emit the per-core Tile program. `ins`/`outs` map name -> bass.AP."""
    xT = ins["xT"]              # [128, 8192] f32   x transposed (full graph)
    xTl = ins["xTl"]            # [128, 1024] f32   x.T local columns
    w1top = ins["w1top"]        # [128, 256] f32
    w1bot = ins["w1bot"]        # [128, 256] f32
    w2bf = ins["w2bf"]          # [256, 256] bf16
    w3blk = ins["w3blk"]        # [128, 2048] bf16  blockdiag W3, host layout
    b1T = ins["b1T"]            # [128, 2] f32
    b2T = ins["b2T"]            # [128, 2] f32
    b3col = ins["b3col"]        # [128, 1] f32
    bcol = ins["bcol"]          # [128, 1] f32
    gidx = ins["gidx"]          # [128, 2048] i16  gather idxs, 8 batches
    tg = ins["tg"]              # [128, 8, 32] f32 t[chosen]
    eg = ins["eg"]              # [128, 8, 32] f32 e_hat[chosen]
    lsidx = ins["lsidx"]        # [128, 8, 2, 32] i16 local_scatter idxs (-1 = skip)
    sarow = ins["sarow"]        # [128, 8, 256] i16 scatter-add token rows
    ypred_out = outs["ypred_out"]   # [1024] f32
    pw_out = outs["pw_out"]         # [8388608] f32

    from contextlib import ExitStack
    stack = ExitStack()

    const = stack.enter_context(tc.tile_pool(name="const", bufs=1))
    psum_mlp = stack.enter_context(tc.tile_pool(name="psum_mlp", bufs=1, space="PSUM"))
    psum_w = stack.enter_context(tc.tile_pool(name="psum_w", bufs=2, space="PSUM"))
    dram = stack.enter_context(tc.tile_pool(name="dram", bufs=1, space="DRAM"))

    # GPSIMD library with the dma_gather ucode must be resident before
    # any InstDMAGatherAnt executes on the Pool engine.
    nc.gpsimd.load_library(library_config.attnmlp)

    # ---------- constant loads ----------
    w1top_sb = const.tile([128, H], F32R)
    nc.sync.dma_start(out=w1top_sb[:], in_=w1top)
    w1bot_sb = const.tile([128, H], F32R)
    nc.sync.dma_start(out=w1bot_sb[:], in_=w1bot)
    w2_sb = const.tile([128, 2, 2, 128], BF16)
    nc.sync.dma_start(
        out=w2_sb[:], in_=w2bf.rearrange("(c1 p) (c2 f) -> p c1 c2 f", p=128, f=128)
    )
    w3_sb = const.tile([128, 2048], BF16)
    nc.sync.dma_start(out=w3_sb[:], in_=w3blk)
    b1T_sb = const.tile([128, 2], F32)
    nc.sync.dma_start(out=b1T_sb[:], in_=b1T)
    b2T_sb = const.tile([128, 2], F32)
    nc.sync.dma_start(out=b2T_sb[:], in_=b2T)
    b3_sb = const.tile([128, 1], F32)
    nc.sync.dma_start(out=b3_sb[:], in_=b3col)
    b_sb = const.tile([128, 1], F32)
    nc.sync.dma_start(out=b_sb[:], in_=bcol)
    gidx_sb = const.tile([128, 2048], I16)
    nc.sync.dma_start(out=gidx_sb[:], in_=gidx)
    tg_sb = const.tile([128, NT, K], F32)
    nc.sync.dma_start(out=tg_sb[:], in_=tg)
    eg_sb = const.tile([128, NT, K], F32)
    nc.sync.dma_start(out=eg_sb[:], in_=eg)
    lsidx_sb = const.tile([128, NT, 2, K], I16)
    nc.sync.dma_start(out=lsidx_sb[:], in_=lsidx)
    sarow_sb = const.tile([128, NT, 256], I16)
    nc.sync.dma_start(out=sarow_sb[:], in_=sarow)

    # ---------- zero-fill of the pairwise block (32 MB dense writes) ----------
    zero_sb = const.tile([128, 4096], F32)
    nc.vector.memzero(zero_sb[:])
    pw_view = pw_out.rearrange("(s p f) -> s p f", p=128, f=4096)  # 16 slabs of 2 MB
    zf_instrs = []
    for s in range(16):
        zf_instrs.append(nc.sync.dma_start(out=pw_view[s], in_=zero_sb[:]))

    # ---------- phase A: A'T (local) and B table (all nodes) ----------
    at_sb = const.tile([128, 2, NL], BF16)       # A'T[h%128, h//128, i]
    b_tab = const.tile([128, N // 128, H], BF16)  # B row j -> part j%128, rank j//128

    with tc.tile_pool(name="xt", bufs=1) as xt_pool:
        xT_sb = xt_pool.tile([128, N], F32R)
        nc.sync.dma_start(out=xT_sb[:], in_=xT)
        xTl_sb = xt_pool.tile([128, NL], F32R)
        nc.sync.dma_start(out=xTl_sb[:], in_=xTl)

        # A'T = (x_loc @ W1top + b1)^T, bf16
        for c in range(2):
            for ib in range(NL // IB):
                ps = psum_w.tile([128, IB], F32, tag="apsum")
                nc.tensor.matmul(
                    out=ps[:],
                    lhsT=w1top_sb[:, c * 128:(c + 1) * 128],
                    rhs=xTl_sb[:, ib * IB:(ib + 1) * IB],
                    start=True, stop=True,
                )
                nc.scalar.activation(
                    at_sb[:, c, ib * IB:(ib + 1) * IB], ps[:],
                    mybir.ActivationFunctionType.Identity,
                    bias=b1T_sb[:, c:c + 1],
                )

        # B = x @ W1bot (all nodes), bf16, laid out for the SBUF gather
        for jc in range(N // 128):
            ps = psum_w.tile([128, H], F32, tag="bpsum")
            nc.tensor.matmul(
                out=ps[:],
                lhsT=xT_sb[:, jc * 128:(jc + 1) * 128],
                rhs=w1bot_sb[:],
                start=True, stop=True,
            )
            nc.vector.tensor_copy(b_tab[:, jc, :], ps[:])

    # ---------- phase B: gather + MLP over 4 batches of 8192 pairs ----------
    # pair q (global) = k * NL + i ; batch b covers k in [8b, 8b+8)
    mlp_ps = psum_mlp.tile([32, NL], F32, space="PSUM")  # [k, i] accumulator

    with tc.tile_pool(name="gather", bufs=2) as gpool, \
         tc.tile_pool(name="work", bufs=3) as work:
        for b in range(GB if ("gather" in PHASES) else 0):
            btg = gpool.tile([128, 2, QB], BF16, tag="btg")
            nc.gpsimd.dma_gather(
                out_ap=btg[:],
                in_ap=b_tab[:].rearrange("p r e -> p (r e)"),
                idxs_ap=gidx_sb[:, b * (QB // 16):(b + 1) * (QB // 16)],
                num_idxs=QB,
                num_idxs_reg=QB,
                elem_size=H,
                transpose=True,
                sbuf_tokens_per_rank=128,
                sbuf_free_dim_per_rank=H * 2,
                single_packet=False,
            )
            for kk in range(QB // NL if ("mlp" in PHASES) else 0):  # k values per batch
                k = b * (QB // NL) + kk
                for ib in range(NL // IB):       # 2 i-blocks
                    q0 = kk * NL + ib * IB
                    i0 = ib * IB
                    # h1T = relu(BT_g + A'T)
                    h1 = work.tile([128, 2, IB], BF16, tag="h1")
                    for c in range(2):
                        nc.vector.tensor_add(
                            h1[:, c, :], btg[:, c, q0:q0 + IB],
                            at_sb[:, c, i0:i0 + IB],
                        )
                        nc.vector.tensor_relu(h1[:, c, :], h1[:, c, :])
                    # h2T = relu(W2^T h1T + b2)
                    h2 = work.tile([128, 2, IB], BF16, tag="h2")
                    for c2 in range(2):
                        ps = psum_w.tile([128, IB], F32, tag="h2psum")
                        for c1 in range(2):
                            nc.tensor.matmul(
                                out=ps[:],
                                lhsT=w2_sb[:, c1, c2, :],
                                rhs=h1[:, c1, :],
                                start=(c1 == 0), stop=(c1 == 1),
                            )
                        nc.scalar.activation(
                            h2[:, c2, :], ps[:],
                            mybir.ActivationFunctionType.Relu,
                            bias=b2T_sb[:, c2:c2 + 1],
                        )
                    # mlp[k, i-block] += W3^T h2T  (block-diagonal lhsT)
                    for c2 in range(2):
                        nc.tensor.matmul(
                            out=mlp_ps[:, i0:i0 + IB],
                            lhsT=w3_sb[:, (c2 * 32 + k) * 32:(c2 * 32 + k) * 32 + 32],
                            rhs=h2[:, c2, :],
                            start=(k == 0 and c2 == 0),
                            stop=(k == 31 and c2 == 1),
                        )

    # ---------- phase C: softmax, weights, Y_pred, scatter ----------
    post = stack.enter_context(tc.tile_pool(name="post", bufs=2))

    mlp_sb = const.tile([32, NL], F32)
    if "mlp" in PHASES:
        nc.scalar.activation(
            mlp_sb[:], mlp_ps[:], mybir.ActivationFunctionType.Identity,
            bias=b3_sb[:32, :],
        )
    else:
        nc.vector.memzero(mlp_sb[:])
    ypred_sb = const.tile([128, NT], F32)
    if "post" not in PHASES or POSTLVL < 4:
        nc.vector.memzero(ypred_sb[:])
    strips_bf = const.tile([128, NT, 2, 1024], BF16)
    if "post" in PHASES and POSTLVL >= 4 and "scatter" in PHASES:
        # Pool stream order: all gathers ran above; switch the Q7 ucode
        # library to the one holding local_scatter.
        nc.gpsimd.load_library(library_config.local_scatter)

    for t in range(NT if ("post" in PHASES) else 0):
        mlpT = post.tile([128, K], F32, tag="mlpT")
        for blk in range(4):
            nc.vector.transpose(
                out=mlpT[blk * 32:(blk + 1) * 32, :],
                in_=mlp_sb[:, t * 128 + blk * 32: t * 128 + (blk + 1) * 32],
            )
        if POSTLVL < 2:
            continue
        ab = post.tile([128, K], F32, tag="ab")
        nc.scalar.activation(ab[:], mlpT[:], mybir.ActivationFunctionType.Abs)
        ex = post.tile([128, K], F32, tag="ex")
        den = post.tile([128, 1], F32, tag="den")
        nc.scalar.activation(
            ex[:], ab[:], mybir.ActivationFunctionType.Exp,
            scale=b_sb[:], accum_out=den[:],
        )
        if POSTLVL < 3:
            continue
        rden = post.tile([128, 1], F32, tag="rden")
        nc.vector.reciprocal(rden[:], den[:])
        w0 = post.tile([128, K], F32, tag="w0")
        nc.vector.tensor_mul(w0[:], mlpT[:], ex[:])
        wt = post.tile([128, K], F32, tag="wt")
        nc.vector.tensor_scalar_mul(wt[:], w0[:], rden[:])
        # Y_pred contribution: sum_k (t - e_hat)[j] * w
        pr = post.tile([128, K], F32, tag="pr")
        nc.vector.tensor_sub(pr[:], tg_sb[:, t, :], eg_sb[:, t, :])
        if POSTLVL < 4:
            continue
        wp = post.tile([128, K], F32, tag="wp")
        nc.vector.tensor_mul(wp[:], wt[:], pr[:])
        nc.vector.reduce_sum(
            out=ypred_sb[:, t:t + 1], in_=wp[:], axis=mybir.AxisListType.X)
        if "scatter" not in PHASES:
            continue
        # Merge this tile's weights into per-slot 64-wide strips via
        # local_scatter (per-partition SBUF scatter; self-pairs and killed
        # duplicates carry idx -1 and are skipped). Slots hold distinct
        # 64-column blocks, so the later scatter-add has no row collisions.
        wt_bf = post.tile([128, K], BF16, tag="wt_bf")
        nc.vector.tensor_copy(wt_bf[:], wt[:])
        for h in range(2):
            nc.gpsimd.local_scatter(
                out_ap=strips_bf[:, t, h, :],
                data_ap=wt_bf[:],
                idxs_ap=lsidx_sb[:, t, h, :],
                channels=128,
                num_elems=1024,
                num_idxs=K,
            )

    if "post" in PHASES and POSTLVL >= 4 and "scatter" in PHASES:
        # Back to the DMA-gather/scatter ucode library for the scatter-adds.
        nc.gpsimd.load_library(library_config.attnmlp)
        for t in range(NT):
            strips_f = post.tile([128, K, 64], F32, tag="strips_f")
            nc.vector.tensor_copy(
                strips_f[:].rearrange("p k e -> p (k e)"), strips_bf[:, t, :, :].rearrange("p h e -> p (h e)"))
            sc = nc.gpsimd.dma_scatter_add(
                out_ap=pw_out.rearrange("(t r e) -> t r e", t=NT, e=64)[t],
                in_ap=strips_f[:],
                idxs_ap=sarow_sb[:, t, :],
                num_idxs=128 * K,
                num_idxs_reg=128 * K,
                elem_size=64,
                single_packet=False,
            )
            for zf in zf_instrs:
                add_dep_helper(sc.ins, zf.ins, reason="scatter after zerofill")

    nc.sync.dma_start(
        out=ypred_out.rearrange("(t p) -> p t", p=128), in_=ypred_sb[:]
    )

    stack.close()


def build_program():
    nc = bacc.Bacc(
        "TRN2", target_bir_lowering=False, debug=False,
        enable_asserts=False, num_devices=NCORES,
        dynamic_dma_scratch_size=65536,
    )
    in_specs = {
        "xT": ([128, N], F32R),
        "xTl": ([128, NL], F32R),
        "w1top": ([128, H], F32R),
        "w1bot": ([128, H], F32R),
        "w2bf": ([H, H], BF16),
        "w3blk": ([128, 2048], BF16),
        "b1T": ([128, 2], F32),
        "b2T": ([128, 2], F32),
        "b3col": ([128, 1], F32),
        "bcol": ([128, 1], F32),
        "gidx": ([128, 2048], I16),
        "tg": ([128, NT, K], F32),
        "eg": ([128, NT, K], F32),
        "lsidx": ([128, NT, 2, K], I16),
        "sarow": ([128, NT, 256], I16),
    }
    out_specs = {
        "ypred_out": ([NL], F32),
        "pw_out": ([NL * N], F32),
    }
    ins = {
        name: nc.dram_tensor(name, shape, dt, kind="ExternalInput").ap()
        for name, (shape, dt) in in_specs.items()
    }
    outs = {
        name: nc.dram_tensor(name, shape, dt, kind="ExternalOutput").ap()
        for name, (shape, dt) in out_specs.items()
    }
    with tile.TileContext(nc) as tc:
        _trace_kernel(nc, tc, ins, outs)
    nc.compile()
    return nc


def host_prep(inputs):
    """Build the 8 per-core input maps from the full problem inputs."""
    x = np.asarray(inputs["x"], np.float32)
    nbrs = np.asarray(inputs["nbrs"])
    t = np.asarray(inputs["t"], np.float32)
    e_hat = np.asarray(inputs["e_hat"], np.float32)
    W1 = np.asarray(inputs["W1"], np.float32)
    b1 = np.asarray(inputs["b1"], np.float32)
    W2 = np.asarray(inputs["W2"], np.float32)
    b2 = np.asarray(inputs["b2"], np.float32)
    W3 = np.asarray(inputs["W3"], np.float32)
    b3 = np.asarray(inputs["b3"], np.float32)
    bscal = float(np.asarray(inputs["b"]))

    chosen = np.ascontiguousarray(nbrs[:, 1:]).astype(np.int32)   # [N, K]
    xT = np.ascontiguousarray(x.T)                                # [128, N]

    w2bf = W2.astype(bf16)
    # blockdiag W3: w3blk[p, (c*32+k)*32 + m] = (m==k) * W3[c*128+p, 0]
    w3blk = np.zeros((128, 2048), np.float32)
    for c in range(2):
        for k in range(32):
            w3blk[:, (c * 32 + k) * 32 + k] = W3[c * 128:(c + 1) * 128, 0]
    w3blk = w3blk.astype(bf16)

    b1T = b1.reshape(2, 128).T.copy()
    b2T = b2.reshape(2, 128).T.copy()
    b3col = np.full((128, 1), float(b3[0]), np.float32)
    bcol = np.full((128, 1), bscal, np.float32)

    in_maps = []
    for c in range(NCORES):
        r0 = c * NL
        ch = chosen[r0:r0 + NL]                                   # [NL, K]
        # gather idxs: q = k*NL + i within each batch of 8 k values
        gidx = np.empty((128, 2048), np.int16)
        for b in range(GB):
            kpb = K // GB
            idxs = ch[:, b * kpb:(b + 1) * kpb].T.reshape(-1)     # q = kk*NL+i
            wrapped = idxs.reshape(QB // 16, 16).T.astype(np.int16)  # [16, QB/16]
            gidx[:, b * (QB // 16):(b + 1) * (QB // 16)] = np.tile(wrapped, (8, 1))
        def tileize(a, dt):
            return np.ascontiguousarray(
                a.reshape(NT, 128, K).transpose(1, 0, 2).astype(dt))
        ivec = np.arange(NL, dtype=np.int64)[:, None]
        # kill self-pairs and all-but-last duplicates within each row
        eqm = ch[:, :, None] == ch[:, None, :]          # [NL, K, K]
        later_dup = (np.triu(eqm, k=1)).any(axis=2)
        killed = later_dup | (ch == (r0 + ivec))        # [NL, K]
        g = (ch >> 6).astype(np.int64)                  # 64-col block index
        u = (ch & 63).astype(np.int64)
        # slot assignment: distinct g per row -> slots 0..S-1; strips live at
        # slot*64+u; scatter-add row for token (p, s) = p*128 + g_of_slot.
        lsidx = np.full((128, NT, 2, K), -1, np.int16)
        sarow = np.empty((128, NT, 256), np.int16)
        for tt in range(NT):
            rowvec = np.empty(128 * K, np.int64)
            for p in range(128):
                i = tt * 128 + p
                slot_of_g = {}
                g_of_slot = []
                for k in range(K):
                    if killed[i, k]:
                        continue
                    gk = g[i, k]
                    if gk not in slot_of_g:
                        slot_of_g[gk] = len(g_of_slot)
                        g_of_slot.append(gk)
                    s = slot_of_g[gk]
                    pos = s * 64 + u[i, k]
                    h = pos >> 10
                    lsidx[p, tt, h, k] = pos - h * 1024
                used = set(g_of_slot)
                dummy = next(gg for gg in range(128) if gg not in used)
                while len(g_of_slot) < K:
                    g_of_slot.append(dummy)
                for s in range(K):
                    rowvec[s * 128 + p] = p * 128 + g_of_slot[s]
            wrapped = rowvec.reshape(256, 16).T.astype(np.int16)
            sarow[:, tt, :] = np.tile(wrapped, (8, 1))
        in_maps.append({
            "xT": xT,
            "xTl": np.ascontiguousarray(xT[:, r0:r0 + NL]),
            "w1top": np.ascontiguousarray(W1[:D]),
            "w1bot": np.ascontiguousarray(W1[D:]),
            "w2bf": w2bf,
            "w3blk": w3blk,
            "b1T": b1T,
            "b2T": b2T,
            "b3col": b3col,
            "bcol": bcol,
            "gidx": gidx,
            "tg": tileize(t[ch], np.float32),
            "eg": tileize(e_hat[ch], np.float32),
            "lsidx": lsidx,
            "sarow": sarow,
        })
    return in_maps


_NC = None


def _get_nc():
    global _NC
    if _NC is None:
        _NC = build_program()
    return _NC


def kernel(_trace=False, _tmpdir=None, **inputs):
    from concourse import bass_utils

    nc = _get_nc()
    in_maps = host_prep(inputs)
    res = bass_utils.run_bass_kernel_spmd(
        nc, in_maps, core_ids=list(range(NCORES)),
        trace=_trace, tmpdir=_tmpdir,
    )
    ypred = np.concatenate([r["ypred_out"] for r in res.results])
    pairwise = np.concatenate(
        [r["pw_out"].reshape(NL, N) for r in res.results], axis=0
    )
    if _trace:
        kernel.last_results = res
    return ypred, pairwise
